# revision 3
# baseline (speedup 1.0000x reference)
"""MultiHeadBatteryAttention on 8 trn2 NeuronCores.

Sharding: data-parallel over batch (2) x tensor-parallel over head groups
(16 heads -> 4 groups of 4).  Core c handles batch c//4, head group c%4.
Each core computes its 4 heads' attention probabilities and its partial
output projection; the host sums the per-group output partials and
concatenates attn along the head axis.

Device math notes:
  - sensor_bias and temporal_bias are constant along the softmax axis, so
    softmax(scores + bias) == softmax(scores): they drop out exactly.
    Masked entries (mask==0) are set to exactly -1e9 in the reference, so
    with an all-ones mask (the spec's fill) the mask is a no-op too.  A
    numpy fallback handles any other mask.
  - Scores are computed twice, in both [q,k] and [k,q] layouts, because the
    PE contracts over partitions: softmax row ops need q on partitions while
    the attn@V matmul needs k on partitions.  exp() runs on ScalarE from
    PSUM in both layouts; row sums come for free from a ones column
    appended to V; the [q,k] side folds normalization into the exp bias:
    attn = exp(s/8 - ln r).
  - All matmuls run in float32r (full PE speed at moving dim >= 256,
    ~1.6e-4 max rel error measured on HW).
"""
import os
import sys

for _p in ("/opt/trn_rl_repo", "/root/.axon_site/_ro/trn_rl_repo"):
    if os.path.isdir(_p) and _p not in sys.path:
        sys.path.insert(0, _p)

import numpy as np

B, S, D, H = 2, 2048, 1024, 16
DK = D // H            # 64
G = 4                  # head groups
HG = H // G            # 4 heads per group
GD = HG * DK           # 256 dims per group
NKT = S // 128         # 16 k (key-position) tiles
NQT = S // 128         # 16 q tiles
NDT = D // 128         # 8 contraction tiles for projections

LAST_RESULT = None     # BassKernelResults of the last device run (for test.py)
_CACHED_NC = None


def _install_tile_patches(tile, mybir):
    """Walrus in this container encodes at most 1 embedded sem wait per
    instruction; Tile emits more.  Hoist excess waits into standalone
    wait_ge instructions on the same engine, and split the kernel-tail
    drain's waits the same way."""
    if getattr(tile.TileContext, "_wait_split_patched", False):
        return

    def _split_waits(tc, inst):
        si = inst.sync_info
        if si is None:
            return
        waits = list(si.on_wait or [])
        if len(waits) <= 1 or tc.sems is None:
            return
        sems = {s.num: s for s in tc.sems.allocated().values()}
        keep, ok = waits[:1], []
        for w in waits[1:]:
            h = sems.get(w.id)
            if h is None or not str(w.wait_mode).startswith("sem-ge"):
                keep.append(w)
                continue
            ok.append((h, w.wait_value))
        if not ok:
            return
        si.on_wait = keep
        eng = tc.nc.engines[inst.engine]
        for h, v in ok:
            eng.wait_ge(h, v)

    def _add_instruction(self, inst):
        if (
            getattr(self, "_lowering_mode", False)
            and inst.engine != mybir.EngineType.Unassigned
        ):
            try:
                _split_waits(self, inst)
            except Exception:
                pass
        self.nc.register_instruction(inst, overwrite=True)
        tile.nn(self.nc.cur_bb).bb.add_instruction(inst)

    def _drain_and_barrier(self, tick_clock, wait_clock):
        from concourse.tile import ScopedClock

        nop = self.nc.sync.nop(nofuse=True, hint="tail_wait_collect")
        wait_clock.add_sem_waits(nop.ins, ScopedClock({None: tick_clock.global_clock}))
        si = nop.ins.sync_info
        waits = list(si.on_wait) if si else []
        if si:
            si.on_wait = []
        sems = {s.num: s for s in self.sems.allocated().values()}
        for w in waits:
            h = sems.get(w.id)
            assert h is not None, (w.ant_name, w.id)
            self.nc.sync.wait_ge(h, w.wait_value)
        self.nc.sync.drain()
        self.nc.all_engine_barrier()
        popped = self.nc._tile_sem_poison_stack.pop()
        assert popped is self._sem_poison
        self.nc.clear_and_free_semaphores(list(self.sems.allocated().values()))
        self.nc.all_engine_barrier()

    tile.TileContext._add_instruction = _add_instruction
    tile.TileContext._drain_and_barrier = _drain_and_barrier
    tile.TileContext._wait_split_patched = True


def _build_nc():
    import concourse.bass as bass
    import concourse.mybir as mybir
    import concourse.tile as tile

    _install_tile_patches(tile, mybir)

    F32, F32R = mybir.dt.float32, mybir.dt.float32r
    AF = mybir.ActivationFunctionType

    nc = bass.Bass("TRN2")
    xt_q = nc.dram_tensor("xt_q", [D, S], F32R, kind="ExternalInput")
    xt_k = nc.dram_tensor("xt_k", [D, S], F32R, kind="ExternalInput")
    xt_v = nc.dram_tensor("xt_v", [D, S], F32R, kind="ExternalInput")
    wq = nc.dram_tensor("wq", [D, GD], F32R, kind="ExternalInput")
    wk = nc.dram_tensor("wk", [D, GD], F32R, kind="ExternalInput")
    wv = nc.dram_tensor("wv", [D, GD], F32R, kind="ExternalInput")
    wo = nc.dram_tensor("wo", [GD, D], F32R, kind="ExternalInput")
    attn_p = nc.dram_tensor("attn_p", [HG, S, S], F32, kind="ExternalOutput")
    out_p = nc.dram_tensor("out_p", [S, D], F32, kind="ExternalOutput")

    with tile.TileContext(nc) as tc:
        with tc.tile_pool(name="persist", bufs=1) as pers, \
             tc.tile_pool(name="usb", bufs=2) as usbp, \
             tc.tile_pool(name="rbp", bufs=2) as rbp, \
             tc.tile_pool(name="work", bufs=4) as work, \
             tc.tile_pool(name="attn_sb", bufs=3) as attnp, \
             tc.tile_pool(name="osb", bufs=2) as osbp, \
             tc.tile_pool(name="dram", bufs=2, space="DRAM") as dpool, \
             tc.tile_pool(name="pp", bufs=1, space="PSUM") as pp, \
             tc.tile_pool(name="pav", bufs=1, space="PSUM") as pav, \
             tc.tile_pool(name="pss", bufs=1, space="PSUM") as pss:

            ones_f = pers.tile([128, 1], F32)
            nc.vector.memset(ones_f, 1.0)
            ones_r = pers.tile([128, 1], F32R)
            nc.vector.tensor_copy(ones_r[:, :], ones_f[:, :])

            QT = [pers.tile([128, S], F32R, tag=f"QT{i}", name=f"QT{i}") for i in range(2)]
            KT = [pers.tile([128, S], F32R, tag=f"KT{i}", name=f"KT{i}") for i in range(2)]
            # V_aug: per s-tile [128, HG*65]; per head 64 V cols + a ones col
            VA = [pers.tile([128, HG * 65], F32R, tag=f"VA{i}", name=f"VA{i}") for i in range(NKT)]
            ctxT = [pers.tile([128, S], F32R, tag=f"ctxT{i}", name=f"ctxT{i}") for i in range(2)]
            wo_sb = [pers.tile([128, D], F32R, tag=f"wo{i}", name=f"wo{i}") for i in range(2)]
            nc.sync.dma_start(out=wo_sb[0], in_=wo[0:128, :])
            nc.sync.dma_start(out=wo_sb[1], in_=wo[128:256, :])

            # ---------------- Phase 1: projections ----------------
            # QT/KT: [gd, s] = W.T-layout proj: lhsT = W k-tile, rhs = XT k-tile
            # V:     [s, gd] natural: lhsT = XT k-tile slice, rhs = Wv k-tile
            for name, w_d, x_d, dst in (("q", wq, xt_q, QT), ("k", wk, xt_k, KT)):
                with tc.tile_pool(name=f"pj{name}", bufs=1) as pj:
                    wt = [pj.tile([128, GD], F32R, tag=f"w{kt}", name=f"wt{kt}") for kt in range(NDT)]
                    for kt in range(NDT):
                        nc.sync.dma_start(out=wt[kt], in_=w_d[kt*128:(kt+1)*128, :])
                    for qp in range(2):
                        xt = [pj.tile([128, 1024], F32R, tag=f"x{kt}", name=f"xtile{name}{qp}{kt}")
                              for kt in range(NDT)]
                        for kt in range(NDT):
                            nc.sync.dma_start(
                                out=xt[kt],
                                in_=x_d[kt*128:(kt+1)*128, qp*1024:(qp+1)*1024])
                        for pt in range(2):
                            ps = pp.tile([128, 1024], F32, tag="pp")
                            for kt in range(NDT):
                                for qc in range(2):
                                    nc.tensor.matmul(
                                        ps[:, qc*512:(qc+1)*512],
                                        wt[kt][:, pt*128:(pt+1)*128],
                                        xt[kt][:, qc*512:(qc+1)*512],
                                        start=(kt == 0), stop=(kt == NDT-1))
                            nc.vector.tensor_copy(
                                dst[pt][:, qp*1024:(qp+1)*1024], ps[:, :])
            with tc.tile_pool(name="pjv", bufs=1) as pj:
                wt = [pj.tile([128, GD], F32R, tag=f"w{kt}", name=f"wvt{kt}") for kt in range(NDT)]
                for kt in range(NDT):
                    nc.sync.dma_start(out=wt[kt], in_=wv[kt*128:(kt+1)*128, :])
                for sh in range(2):
                    xt = [pj.tile([128, 1024], F32R, tag=f"x{kt}", name=f"xtilev{sh}{kt}")
                          for kt in range(NDT)]
                    for kt in range(NDT):
                        nc.sync.dma_start(
                            out=xt[kt],
                            in_=xt_v[kt*128:(kt+1)*128, sh*1024:(sh+1)*1024])
                    for sti in range(8):
                        st = sh * 8 + sti
                        ps = pp.tile([128, 1024], F32, tag="pp")
                        for kt in range(NDT):
                            nc.tensor.matmul(
                                ps[:, 0:GD],
                                xt[kt][:, sti*128:(sti+1)*128],
                                wt[kt][:, :],
                                start=(kt == 0), stop=(kt == NDT-1))
                        va = VA[st]
                        va_v = va[:, :].rearrange("p (h c) -> p h c", h=HG)
                        nc.vector.tensor_copy(
                            va_v[:, :, 0:64],
                            ps[:, 0:GD].rearrange("p (h c) -> p h c", h=HG))
                        for h in range(HG):
                            nc.vector.tensor_copy(va[:, h*65+64:h*65+65], ones_r[:, :])

            # ---------------- Phase 2: attention per head ----------------
            for h in range(HG):
                pt, po = h // 2, (h % 2) * 64
                qt_h = QT[pt]
                kt_h = KT[pt]

                # S.T (scores transposed) -> exp -> E.T ; AV accumulate
                uacc = pav.tile([65, S], F32, tag="uacc")
                for kt in range(NKT):
                    for qh in range(2):
                        stp = pp.tile([128, 1024], F32, tag="pp")
                        for qc in range(2):
                            nc.tensor.matmul(
                                stp[:, qc*512:(qc+1)*512],
                                kt_h[po:po+DK, kt*128:(kt+1)*128],
                                qt_h[po:po+DK, qh*1024+qc*512:qh*1024+(qc+1)*512],
                                start=True, stop=True)
                        et = work.tile([128, 1024], F32R, tag="et")
                        nc.scalar.activation(et[:, :], stp[:, :], AF.Exp,
                                             bias=0.0, scale=0.125)
                        for qc in range(2):
                            nc.tensor.matmul(
                                uacc[:, qh*1024+qc*512:qh*1024+(qc+1)*512],
                                VA[kt][:, h*65:(h+1)*65],
                                et[:, qc*512:(qc+1)*512],
                                start=(kt == 0), stop=(kt == NKT-1))
                usb = usbp.tile([65, S], F32, tag="usb")
                nc.vector.tensor_copy(usb[:, :], uacc[:, :])

                # r plumbing: r row -> DRAM -> rT [128, NQT] and rb [64, S]
                r_dram = dpool.tile([1, S], F32, tag="r")
                nc.sync.dma_start(out=r_dram[:, :], in_=usb[64:65, :])
                rT = rbp.tile([128, NQT], F32, tag="rT")
                rsrc = nc.bass_ap(r_dram, ap=[[1, 128], [128, NQT]])
                nc.sync.dma_start(out=rT[:, :], in_=rsrc)
                nlnrT = rbp.tile([128, NQT], F32, tag="nlnrT")
                nc.scalar.activation(nlnrT[:, :], rT[:, :], AF.Ln,
                                     bias=0.0, scale=1.0)
                nc.vector.tensor_scalar_mul(nlnrT[:, :], nlnrT[:, :], -1.0)

                rb = rbp.tile([64, S], F32, tag="rb")
                bsrc = nc.bass_ap(r_dram, ap=[[0, 64], [1, S]])
                nc.gpsimd.dma_start(out=rb[:, :], in_=bsrc)
                nc.vector.reciprocal(rb[:, :], rb[:, :])
                nc.vector.tensor_mul(ctxT[pt][po:po+64, :], rb[:, :], usb[0:64, :])

                # S side: normalized attn = exp(s/8 - ln r) -> HBM
                for qt in range(NQT):
                    at = attnp.tile([128, S], F32, tag="at")
                    for kh in range(2):
                        sp = pss.tile([128, 1024], F32, tag="pss")
                        for kc in range(2):
                            nc.tensor.matmul(
                                sp[:, kc*512:(kc+1)*512],
                                qt_h[po:po+DK, qt*128:(qt+1)*128],
                                kt_h[po:po+DK, kh*1024+kc*512:kh*1024+(kc+1)*512],
                                start=True, stop=True)
                        nc.scalar.activation(at[:, kh*1024:(kh+1)*1024], sp[:, :],
                                             AF.Exp, bias=nlnrT[:, qt:qt+1],
                                             scale=0.125)
                    nc.sync.dma_start(out=attn_p[h, qt*128:(qt+1)*128, :],
                                      in_=at[:, :])

            # ---------------- Phase 3: output projection ----------------
            for qt in range(NQT):
                ps = pp.tile([128, 1024], F32, tag="pp")
                for pair in range(2):
                    for nch in range(2):
                        nc.tensor.matmul(
                            ps[:, nch*512:(nch+1)*512],
                            ctxT[pair][:, qt*128:(qt+1)*128],
                            wo_sb[pair][:, nch*512:(nch+1)*512],
                            start=(pair == 0), stop=(pair == 1))
                ob = osbp.tile([128, 1024], F32, tag="ob")
                nc.vector.tensor_copy(ob[:, :], ps[:, :])
                nc.sync.dma_start(out=out_p[qt*128:(qt+1)*128, :], in_=ob[:, :])
    return nc


def _bass_ap_helper():
    """Attach a small helper to Bass for raw APs over dram tiles."""
    import concourse.bass as bass

    def bass_ap(self, dram_tile, ap):
        v = dram_tile[:, :]
        return bass.AP(tensor=v.tensor, offset=v.offset, ap=ap)

    bass.Bass.bass_ap = bass_ap


def _numpy_fallback(query, key, value, mask, sensor_weights, Wq, Wk, Wv, Wo, bo,
                    sensor_attention, temporal_bias):
    out = np.empty((B, S, D), np.float32)
    attn = np.empty((B, H, S, S), np.float32)
    scale = np.sqrt(np.float32(DK))
    for b in range(B):
        Q = (query[b] @ Wq).reshape(S, H, DK).transpose(1, 0, 2)
        K = (key[b] @ Wk).reshape(S, H, DK).transpose(1, 0, 2)
        V = (value[b] @ Wv).reshape(S, H, DK).transpose(1, 0, 2)
        sb = sensor_attention @ sensor_weights[b].T  # [H, S]
        ctx = np.empty((S, H, DK), np.float32)
        for h in range(H):
            s = Q[h] @ K[h].T / scale + temporal_bias[0, h, 0, 0]
            s = s + sb[h][:, None]
            s = np.where(mask[b, 0] == 0, np.float32(-1e9), s)
            s = s - s.max(axis=1, keepdims=True)
            e = np.exp(s)
            a = e / e.sum(axis=1, keepdims=True)
            attn[b, h] = a
            ctx[:, h, :] = a @ V[h]
        out[b] = ctx.reshape(S, D) @ Wo + bo
    return out, attn


def kernel(**inputs):
    global LAST_RESULT, _CACHED_NC

    inp = {k: np.asarray(v) for k, v in inputs.items()}
    query, key, value = inp["query"], inp["key"], inp["value"]
    mask = inp["mask"]
    Wq, Wk, Wv, Wo, bo = inp["Wq"], inp["Wk"], inp["Wv"], inp["Wo"], inp["bo"]

    expected = (query.shape == (B, S, D) and key.shape == (B, S, D)
                and value.shape == (B, S, D) and Wq.shape == (D, D)
                and mask.shape == (B, 1, S, S))
    if not expected or not np.all(mask != 0):
        return _numpy_fallback(
            query.astype(np.float32), key.astype(np.float32),
            value.astype(np.float32), mask, inp["sensor_weights"],
            Wq.astype(np.float32), Wk.astype(np.float32),
            Wv.astype(np.float32), Wo.astype(np.float32),
            bo.astype(np.float32), inp["sensor_attention"],
            inp["temporal_bias"])

    from concourse.bass_utils import run_bass_kernel_spmd

    _bass_ap_helper()
    if _CACHED_NC is None:
        _CACHED_NC = _build_nc()
    nc = _CACHED_NC

    f32 = np.float32
    xt = {b: {n: np.ascontiguousarray(a[b].T.astype(f32))
              for n, a in (("q", query), ("k", key), ("v", value))}
          for b in range(B)}
    wsl = {g: {
        "wq": np.ascontiguousarray(Wq[:, g*GD:(g+1)*GD].astype(f32)),
        "wk": np.ascontiguousarray(Wk[:, g*GD:(g+1)*GD].astype(f32)),
        "wv": np.ascontiguousarray(Wv[:, g*GD:(g+1)*GD].astype(f32)),
        "wo": np.ascontiguousarray(Wo[g*GD:(g+1)*GD, :].astype(f32)),
    } for g in range(G)}

    in_maps = []
    for c in range(8):
        b, g = c // G, c % G
        in_maps.append({
            "xt_q": xt[b]["q"], "xt_k": xt[b]["k"], "xt_v": xt[b]["v"],
            **wsl[g],
        })

    res = run_bass_kernel_spmd(
        nc, in_maps, core_ids=list(range(8)),
        trace_cores=list(range(8)) if os.environ.get("BASS_TRACE") else None,
    )
    LAST_RESULT = res

    output = np.empty((B, S, D), np.float32)
    attn = np.empty((B, H, S, S), np.float32)
    for c in range(8):
        b, g = c // G, c % G
        attn[b, g*HG:(g+1)*HG] = res.results[c]["attn_p"]
    for b in range(B):
        acc = res.results[b*G]["out_p"].copy()
        for g in range(1, G):
            acc += res.results[b*G + g]["out_p"]
        output[b] = acc + bo.astype(np.float32)
    return output, attn


# revision 4
# speedup vs baseline: 1.0364x; 1.0364x over previous
"""MultiHeadBatteryAttention on 8 trn2 NeuronCores.

Sharding: data-parallel over batch (2) x tensor-parallel over head groups
(16 heads -> 4 groups of 4).  Core c handles batch c//4, head group c%4.
Each core computes its 4 heads' attention probabilities and its partial
output projection; the host sums the per-group output partials and
concatenates attn along the head axis.

Device math notes:
  - sensor_bias and temporal_bias are constant along the softmax axis, so
    softmax(scores + bias) == softmax(scores): they drop out exactly.
    Masked entries (mask==0) are set to exactly -1e9 in the reference, so
    with an all-ones mask (the spec's fill) the mask is a no-op too.  A
    numpy fallback handles any other mask.
  - Scores are computed twice, in both [q,k] and [k,q] layouts, because the
    PE contracts over partitions: softmax row ops need q on partitions while
    the attn@V matmul needs k on partitions.  exp() runs on ScalarE from
    PSUM in both layouts; row sums come for free from a ones column
    appended to V; the [q,k] side folds normalization into the exp bias:
    attn = exp(s/8 - ln r).
  - All matmuls run in float32r (full PE speed at moving dim >= 256,
    ~1.6e-4 max rel error measured on HW).
"""
import os
import sys

for _p in ("/opt/trn_rl_repo", "/root/.axon_site/_ro/trn_rl_repo"):
    if os.path.isdir(_p) and _p not in sys.path:
        sys.path.insert(0, _p)

import numpy as np

B, S, D, H = 2, 2048, 1024, 16
DK = D // H            # 64
G = 4                  # head groups
HG = H // G            # 4 heads per group
GD = HG * DK           # 256 dims per group
NKT = S // 128         # 16 k (key-position) tiles
NQT = S // 128         # 16 q tiles
NDT = D // 128         # 8 contraction tiles for projections

LAST_RESULT = None     # BassKernelResults of the last device run (for test.py)
_CACHED_NC = None


def _install_tile_patches(tile, mybir):
    """Walrus in this container encodes at most 1 embedded sem wait per
    instruction; Tile emits more.  Hoist excess waits into standalone
    wait_ge instructions on the same engine, and split the kernel-tail
    drain's waits the same way."""
    if getattr(tile.TileContext, "_wait_split_patched", False):
        return

    def _split_waits(tc, inst):
        si = inst.sync_info
        if si is None:
            return
        waits = list(si.on_wait or [])
        if len(waits) <= 1 or tc.sems is None:
            return
        sems = {s.num: s for s in tc.sems.allocated().values()}
        keep, ok = waits[:1], []
        for w in waits[1:]:
            h = sems.get(w.id)
            if h is None or not str(w.wait_mode).startswith("sem-ge"):
                keep.append(w)
                continue
            ok.append((h, w.wait_value))
        if not ok:
            return
        si.on_wait = keep
        eng = tc.nc.engines[inst.engine]
        for h, v in ok:
            eng.wait_ge(h, v)

    def _add_instruction(self, inst):
        if (
            getattr(self, "_lowering_mode", False)
            and inst.engine != mybir.EngineType.Unassigned
        ):
            try:
                _split_waits(self, inst)
            except Exception:
                pass
        self.nc.register_instruction(inst, overwrite=True)
        tile.nn(self.nc.cur_bb).bb.add_instruction(inst)

    def _drain_and_barrier(self, tick_clock, wait_clock):
        from concourse.tile import ScopedClock

        nop = self.nc.sync.nop(nofuse=True, hint="tail_wait_collect")
        wait_clock.add_sem_waits(nop.ins, ScopedClock({None: tick_clock.global_clock}))
        si = nop.ins.sync_info
        waits = list(si.on_wait) if si else []
        if si:
            si.on_wait = []
        sems = {s.num: s for s in self.sems.allocated().values()}
        for w in waits:
            h = sems.get(w.id)
            assert h is not None, (w.ant_name, w.id)
            self.nc.sync.wait_ge(h, w.wait_value)
        self.nc.sync.drain()
        self.nc.all_engine_barrier()
        popped = self.nc._tile_sem_poison_stack.pop()
        assert popped is self._sem_poison
        self.nc.clear_and_free_semaphores(list(self.sems.allocated().values()))
        self.nc.all_engine_barrier()

    tile.TileContext._add_instruction = _add_instruction
    tile.TileContext._drain_and_barrier = _drain_and_barrier
    tile.TileContext._wait_split_patched = True


def _build_nc():
    import concourse.bass as bass
    import concourse.mybir as mybir
    import concourse.tile as tile

    _install_tile_patches(tile, mybir)

    F32, F32R = mybir.dt.float32, mybir.dt.float32r
    BF16 = mybir.dt.bfloat16
    AF = mybir.ActivationFunctionType

    nc = bass.Bass("TRN2")
    xt_q = nc.dram_tensor("xt_q", [D, S], F32R, kind="ExternalInput")
    xt_k = nc.dram_tensor("xt_k", [D, S], F32R, kind="ExternalInput")
    xt_v = nc.dram_tensor("xt_v", [D, S], F32R, kind="ExternalInput")
    wq = nc.dram_tensor("wq", [D, GD], F32R, kind="ExternalInput")
    wk = nc.dram_tensor("wk", [D, GD], F32R, kind="ExternalInput")
    wv = nc.dram_tensor("wv", [D, GD], F32R, kind="ExternalInput")
    wo = nc.dram_tensor("wo", [GD, D], F32R, kind="ExternalInput")
    attn_p = nc.dram_tensor("attn_p", [HG, S, S], F32, kind="ExternalOutput")
    out_p = nc.dram_tensor("out_p", [S, D], F32, kind="ExternalOutput")

    with tile.TileContext(nc) as tc:
        with tc.tile_pool(name="persist", bufs=1) as pers, \
             tc.tile_pool(name="usb", bufs=2) as usbp, \
             tc.tile_pool(name="rbp", bufs=2) as rbp, \
             tc.tile_pool(name="work", bufs=6) as work, \
             tc.tile_pool(name="attn_sb", bufs=3) as attnp, \
             tc.tile_pool(name="osb", bufs=2) as osbp, \
             tc.tile_pool(name="dram", bufs=2, space="DRAM") as dpool, \
             tc.tile_pool(name="pp", bufs=1, space="PSUM") as pp, \
             tc.tile_pool(name="pav", bufs=1, space="PSUM") as pav, \
             tc.tile_pool(name="pss", bufs=1, space="PSUM") as pss:

            ones_f = pers.tile([128, 1], F32)
            nc.vector.memset(ones_f, 1.0)
            ones_r = pers.tile([128, 1], BF16)
            nc.vector.tensor_copy(ones_r[:, :], ones_f[:, :])

            QT = [pers.tile([128, S], BF16, tag=f"QT{i}", name=f"QT{i}") for i in range(2)]
            KT = [pers.tile([128, S], BF16, tag=f"KT{i}", name=f"KT{i}") for i in range(2)]
            # V_aug: per s-tile [128, HG*65]; per head 64 V cols + a ones col
            VA = [pers.tile([128, HG * 65], BF16, tag=f"VA{i}", name=f"VA{i}") for i in range(NKT)]
            ctxT = [pers.tile([128, S], F32R, tag=f"ctxT{i}", name=f"ctxT{i}") for i in range(2)]
            wo_sb = [pers.tile([128, D], F32R, tag=f"wo{i}", name=f"wo{i}") for i in range(2)]
            nc.sync.dma_start(out=wo_sb[0], in_=wo[0:128, :])
            nc.sync.dma_start(out=wo_sb[1], in_=wo[128:256, :])

            # ---------------- Phase 1: projections ----------------
            # QT/KT: [gd, s] = W.T-layout proj: lhsT = W k-tile, rhs = XT k-tile
            # V:     [s, gd] natural: lhsT = XT k-tile slice, rhs = Wv k-tile
            for name, w_d, x_d, dst in (("q", wq, xt_q, QT), ("k", wk, xt_k, KT)):
                with tc.tile_pool(name=f"pj{name}", bufs=1) as pj:
                    wt = [pj.tile([128, GD], F32R, tag=f"w{kt}", name=f"wt{kt}") for kt in range(NDT)]
                    for kt in range(NDT):
                        nc.sync.dma_start(out=wt[kt], in_=w_d[kt*128:(kt+1)*128, :])
                    for qp in range(2):
                        xt = [pj.tile([128, 1024], F32R, tag=f"x{kt}", name=f"xtile{name}{qp}{kt}")
                              for kt in range(NDT)]
                        for kt in range(NDT):
                            nc.sync.dma_start(
                                out=xt[kt],
                                in_=x_d[kt*128:(kt+1)*128, qp*1024:(qp+1)*1024])
                        for pt in range(2):
                            ps = pp.tile([128, 1024], F32, tag="pp")
                            for kt in range(NDT):
                                for qc in range(2):
                                    nc.tensor.matmul(
                                        ps[:, qc*512:(qc+1)*512],
                                        wt[kt][:, pt*128:(pt+1)*128],
                                        xt[kt][:, qc*512:(qc+1)*512],
                                        start=(kt == 0), stop=(kt == NDT-1))
                            nc.vector.tensor_copy(
                                dst[pt][:, qp*1024:(qp+1)*1024], ps[:, :])
            with tc.tile_pool(name="pjv", bufs=1) as pj:
                wt = [pj.tile([128, GD], F32R, tag=f"w{kt}", name=f"wvt{kt}") for kt in range(NDT)]
                for kt in range(NDT):
                    nc.sync.dma_start(out=wt[kt], in_=wv[kt*128:(kt+1)*128, :])
                for sh in range(2):
                    xt = [pj.tile([128, 1024], F32R, tag=f"x{kt}", name=f"xtilev{sh}{kt}")
                          for kt in range(NDT)]
                    for kt in range(NDT):
                        nc.sync.dma_start(
                            out=xt[kt],
                            in_=xt_v[kt*128:(kt+1)*128, sh*1024:(sh+1)*1024])
                    for sti in range(8):
                        st = sh * 8 + sti
                        ps = pp.tile([128, 1024], F32, tag="pp")
                        for kt in range(NDT):
                            nc.tensor.matmul(
                                ps[:, 0:GD],
                                xt[kt][:, sti*128:(sti+1)*128],
                                wt[kt][:, :],
                                start=(kt == 0), stop=(kt == NDT-1))
                        va = VA[st]
                        va_v = va[:, :].rearrange("p (h c) -> p h c", h=HG)
                        nc.vector.tensor_copy(
                            va_v[:, :, 0:64],
                            ps[:, 0:GD].rearrange("p (h c) -> p h c", h=HG))
                        for h in range(HG):
                            nc.vector.tensor_copy(va[:, h*65+64:h*65+65], ones_r[:, :])

            # ---------------- Phase 2: attention per head ----------------
            for h in range(HG):
                pt, po = h // 2, (h % 2) * 64
                qt_h = QT[pt]
                kt_h = KT[pt]

                # S.T (scores transposed) -> exp -> E.T ; AV accumulate
                uacc = pav.tile([65, S], F32, tag="uacc")
                for kt in range(NKT):
                    for qh in range(2):
                        stp = pp.tile([128, 1024], F32, tag="pp")
                        for qc in range(2):
                            nc.tensor.matmul(
                                stp[:, qc*512:(qc+1)*512],
                                kt_h[po:po+DK, kt*128:(kt+1)*128],
                                qt_h[po:po+DK, qh*1024+qc*512:qh*1024+(qc+1)*512],
                                start=True, stop=True)
                        et = work.tile([128, 1024], BF16, tag="et", bufs=6)
                        nc.scalar.activation(et[:, :], stp[:, :], AF.Exp,
                                             bias=0.0, scale=0.125)
                        for qc in range(2):
                            nc.tensor.matmul(
                                uacc[:, qh*1024+qc*512:qh*1024+(qc+1)*512],
                                VA[kt][:, h*65:(h+1)*65],
                                et[:, qc*512:(qc+1)*512],
                                start=(kt == 0), stop=(kt == NKT-1))
                usb = usbp.tile([65, S], F32, tag="usb")
                nc.vector.tensor_copy(usb[:, :], uacc[:, :])

                # r plumbing: r row -> DRAM -> rT [128, NQT] and rb [64, S]
                r_dram = dpool.tile([1, S], F32, tag="r")
                nc.sync.dma_start(out=r_dram[:, :], in_=usb[64:65, :])
                rT = rbp.tile([128, NQT], F32, tag="rT")
                rsrc = nc.bass_ap(r_dram, ap=[[1, 128], [128, NQT]])
                nc.sync.dma_start(out=rT[:, :], in_=rsrc)
                nlnrT = rbp.tile([128, NQT], F32, tag="nlnrT")
                nc.scalar.activation(nlnrT[:, :], rT[:, :], AF.Ln,
                                     bias=0.0, scale=1.0)
                nc.vector.tensor_scalar_mul(nlnrT[:, :], nlnrT[:, :], -1.0)

                recipT = rbp.tile([128, NQT], F32, tag="recipT")
                nc.vector.reciprocal(recipT[:, :], rT[:, :])
                rrec_dram = dpool.tile([1, S], F32, tag="rrec")
                rrdst = nc.bass_ap(rrec_dram, ap=[[1, 128], [128, NQT]])
                nc.sync.dma_start(out=rrdst, in_=recipT[:, :])
                rb = rbp.tile([64, S], F32, tag="rb")
                bsrc = nc.bass_ap(rrec_dram, ap=[[0, 64], [1, S]])
                nc.gpsimd.dma_start(out=rb[:, :], in_=bsrc)
                nc.vector.tensor_mul(ctxT[pt][po:po+64, :], rb[:, :], usb[0:64, :])

                # S side: normalized attn = exp(s/8 - ln r) -> HBM
                for qt in range(NQT):
                    at = attnp.tile([128, S], F32, tag="at")
                    for kh in range(2):
                        sp = pss.tile([128, 1024], F32, tag="pss")
                        for kc in range(2):
                            nc.tensor.matmul(
                                sp[:, kc*512:(kc+1)*512],
                                qt_h[po:po+DK, qt*128:(qt+1)*128],
                                kt_h[po:po+DK, kh*1024+kc*512:kh*1024+(kc+1)*512],
                                start=True, stop=True)
                        nc.scalar.activation(at[:, kh*1024:(kh+1)*1024], sp[:, :],
                                             AF.Exp, bias=nlnrT[:, qt:qt+1],
                                             scale=0.125)
                    nc.sync.dma_start(out=attn_p[h, qt*128:(qt+1)*128, :],
                                      in_=at[:, :])

            # ---------------- Phase 3: output projection ----------------
            for qt in range(NQT):
                ps = pp.tile([128, 1024], F32, tag="pp")
                for pair in range(2):
                    for nch in range(2):
                        nc.tensor.matmul(
                            ps[:, nch*512:(nch+1)*512],
                            ctxT[pair][:, qt*128:(qt+1)*128],
                            wo_sb[pair][:, nch*512:(nch+1)*512],
                            start=(pair == 0), stop=(pair == 1))
                ob = osbp.tile([128, 1024], F32, tag="ob")
                nc.vector.tensor_copy(ob[:, :], ps[:, :])
                nc.sync.dma_start(out=out_p[qt*128:(qt+1)*128, :], in_=ob[:, :])
    return nc


def _bass_ap_helper():
    """Attach a small helper to Bass for raw APs over dram tiles."""
    import concourse.bass as bass

    def bass_ap(self, dram_tile, ap):
        v = dram_tile[:, :]
        return bass.AP(tensor=v.tensor, offset=v.offset, ap=ap)

    bass.Bass.bass_ap = bass_ap


def _numpy_fallback(query, key, value, mask, sensor_weights, Wq, Wk, Wv, Wo, bo,
                    sensor_attention, temporal_bias):
    out = np.empty((B, S, D), np.float32)
    attn = np.empty((B, H, S, S), np.float32)
    scale = np.sqrt(np.float32(DK))
    for b in range(B):
        Q = (query[b] @ Wq).reshape(S, H, DK).transpose(1, 0, 2)
        K = (key[b] @ Wk).reshape(S, H, DK).transpose(1, 0, 2)
        V = (value[b] @ Wv).reshape(S, H, DK).transpose(1, 0, 2)
        sb = sensor_attention @ sensor_weights[b].T  # [H, S]
        ctx = np.empty((S, H, DK), np.float32)
        for h in range(H):
            s = Q[h] @ K[h].T / scale + temporal_bias[0, h, 0, 0]
            s = s + sb[h][:, None]
            s = np.where(mask[b, 0] == 0, np.float32(-1e9), s)
            s = s - s.max(axis=1, keepdims=True)
            e = np.exp(s)
            a = e / e.sum(axis=1, keepdims=True)
            attn[b, h] = a
            ctx[:, h, :] = a @ V[h]
        out[b] = ctx.reshape(S, D) @ Wo + bo
    return out, attn


def kernel(**inputs):
    global LAST_RESULT, _CACHED_NC

    inp = {k: np.asarray(v) for k, v in inputs.items()}
    query, key, value = inp["query"], inp["key"], inp["value"]
    mask = inp["mask"]
    Wq, Wk, Wv, Wo, bo = inp["Wq"], inp["Wk"], inp["Wv"], inp["Wo"], inp["bo"]

    expected = (query.shape == (B, S, D) and key.shape == (B, S, D)
                and value.shape == (B, S, D) and Wq.shape == (D, D)
                and mask.shape == (B, 1, S, S))
    if not expected or not np.all(mask != 0):
        return _numpy_fallback(
            query.astype(np.float32), key.astype(np.float32),
            value.astype(np.float32), mask, inp["sensor_weights"],
            Wq.astype(np.float32), Wk.astype(np.float32),
            Wv.astype(np.float32), Wo.astype(np.float32),
            bo.astype(np.float32), inp["sensor_attention"],
            inp["temporal_bias"])

    from concourse.bass_utils import run_bass_kernel_spmd

    _bass_ap_helper()
    if _CACHED_NC is None:
        _CACHED_NC = _build_nc()
    nc = _CACHED_NC

    f32 = np.float32
    xt = {b: {n: np.ascontiguousarray(a[b].T.astype(f32))
              for n, a in (("q", query), ("k", key), ("v", value))}
          for b in range(B)}
    wsl = {g: {
        "wq": np.ascontiguousarray(Wq[:, g*GD:(g+1)*GD].astype(f32)),
        "wk": np.ascontiguousarray(Wk[:, g*GD:(g+1)*GD].astype(f32)),
        "wv": np.ascontiguousarray(Wv[:, g*GD:(g+1)*GD].astype(f32)),
        "wo": np.ascontiguousarray(Wo[g*GD:(g+1)*GD, :].astype(f32)),
    } for g in range(G)}

    in_maps = []
    for c in range(8):
        b, g = c // G, c % G
        in_maps.append({
            "xt_q": xt[b]["q"], "xt_k": xt[b]["k"], "xt_v": xt[b]["v"],
            **wsl[g],
        })

    res = run_bass_kernel_spmd(
        nc, in_maps, core_ids=list(range(8)),
        trace_cores=list(range(8)) if os.environ.get("BASS_TRACE") else None,
    )
    LAST_RESULT = res

    output = np.empty((B, S, D), np.float32)
    attn = np.empty((B, H, S, S), np.float32)
    for c in range(8):
        b, g = c // G, c % G
        attn[b, g*HG:(g+1)*HG] = res.results[c]["attn_p"]
    for b in range(B):
        acc = res.results[b*G]["out_p"].copy()
        for g in range(1, G):
            acc += res.results[b*G + g]["out_p"]
        output[b] = acc + bo.astype(np.float32)
    return output, attn
